# revision 4
# baseline (speedup 1.0000x reference)
"""Trainium2 Bass kernel for nn_AdditiveAttention (Bahdanau attention).

Reference computation (B=16, Q=128, K=128, D=512, H=512):
    q = queries @ Wq                     [B,Q,H]
    k = keys @ Wk                        [B,K,H]
    scores[b,q,k] = sum_h wv[h] * tanh(q[b,q,h] + k[b,k,h])
    attn = softmax over valid keys (k < valid_lens[b])
    out = attn @ values                  [B,Q,D]

Strategy (8 NeuronCores, SPMD, key-split data parallelism):
  Each batch's valid key range is split into contiguous fragments packed
  into 8 cores x S uniform slots (see _plan/_pack).  A cell computes the
  UNNORMALIZED partial o = exp(scores) @ values and z = sum(exp(scores));
  the host combines out[b] = sum(o) / sum(z).

  The per-key work sum_h wv_h * tanh(qp_h + kp_h) is elementwise-bound
  (H*Q = 65536 elements per key).  To beat the single-engine limit the
  keys of each slot are SPLIT across two compute paths:
    * scalar path: broadcast-add on DVE (2x_1P packed bf16) or GpSimd,
      then exact tanh on ScalarE in big batched instructions.
    * dve path: ONE custom DVE instruction per (group, h-chunk) that
      fuses the broadcast add with a clamped-cubic tanh approximation
      f(u) = m*(1 + c1*m^2), m = clamp(u, +-c0); the global scale K of
      the fit K*f ~ tanh is folded into a second copy of wv.
  The wv reduction runs on TensorE (per key: 4 accumulated [128h x 128q]
  x [128h x 1] matmuls into one PSUM score column); masked exp on
  ScalarE; transpose + exp@values on TensorE.
"""

import os
import sys
import types
import math
import bisect
import numpy as np
import ml_dtypes

# ---------------------------------------------------------------------------
# axon NTFF profile hook (lets trace=True / BASS_TRACE=1 work in this image)
# ---------------------------------------------------------------------------
def _install_axon_hooks():
    if "antenv.axon_hooks" in sys.modules:
        return
    try:
        import trn_agent_boot.trn_boot as _tb

        _hooks = types.ModuleType("antenv.axon_hooks")
        _hook = _tb._ntff_profile_via_ctypes("/opt/axon/libaxon_pjrt.so")
        _hooks.get_axon_ntff_profile_hook = lambda: _hook
        _hooks.set_axon_ntff_profile_hook = lambda h: None
        sys.modules["antenv.axon_hooks"] = _hooks
    except Exception:
        pass


_install_axon_hooks()

import concourse.bass as bass
import concourse.bacc as bacc
import concourse.mybir as mybir
import concourse.tile as tile
import concourse.bass_utils as bass_utils
from concourse.bass_utils import run_bass_kernel_spmd
from concourse.masks import make_identity

# Avoid S3 artifact-upload attempts in the trace path.
bass_utils.upload_artifacts = lambda tmpdir: tmpdir

F32 = mybir.dt.float32
BF16 = mybir.dt.bfloat16
BF16_NP = ml_dtypes.bfloat16

B, Q, K, D, H = 16, 128, 128, 512, 512
NCORES = 8
KT = 16  # key-columns per tanh group
NEG = -1e9

# clamped-cubic tanh fit: tanh(u) ~ KV * m * (1 + C1V*m^2), m = clip(u, +-C0V)
# (density-weighted least squares against u ~ N(0, 1.42), the empirical
# distribution of q_h + k_h for this problem's scale)
C0V, C1V, KV = 1.59679101, -0.13073221, 0.89431216

# tunables
DVE_FRAC = 0.30   # fraction of each slot's keys on the custom-DVE tanh path
GP_GROUPS = 2     # scalar-path add-groups offloaded to GpSimd (biggest slots)

_NC_CACHE: dict = {}
LAST_RESULT = None


# ---------------------------------------------------------------------------
# custom DVE op: fused broadcast-add + clamped-cubic tanh
# ---------------------------------------------------------------------------
def _register_tanh_op():
    import concourse.dve_ops as dve_ops
    from concourse.dve_ops import DveOp
    from concourse.dve_spec import Spec, Src0, Src1, C0, C1, Zero, One, maxx, minn
    from concourse.dve_spec import lower
    from concourse.dve_uop import DveOpSpec

    name = "TANH_BAHDANAU_ANT"
    if name in dve_ops._SUB_OPCODE_FOR_NAME:
        return next(op for op in dve_ops.OPS if op.name == name)
    u = Src0 + Src1
    m = maxx(minn(u, C0), Zero - C0)
    v = m * m
    body = m * ((v * C1) + One)

    def ref(in0, in1, s0, s1, imm2):
        mm = np.clip(in0.astype(np.float32) + in1, -s0, s0)
        return (mm * (1.0 + s1 * mm * mm)).astype(np.float32)

    spec = Spec(body=body, reference=ref)
    row = max(dve_ops._SUB_OPCODE_FOR_NAME.values()) + 1
    assert row < 0x20
    dve_ops._SUB_OPCODE_FOR_NAME[name] = row
    ver = "v3"
    tmp = DveOpSpec(name=name, opcode=row, uops=lower(spec, ver=ver), rd1_en=True)
    op = DveOp(name, spec, subdim=False, uops_sha={ver: tmp.sha(ver)})
    dve_ops.OPS.append(op)
    dve_ops.CUSTOM_DVE_SPECS[name] = spec
    return op


TANH_OP = _register_tanh_op()


def _pack(vl, caps):
    """Pack each batch's valid keys as contiguous ranges into cells (one
    range per cell).  Best-fit: smallest cell that fits the remainder,
    else the largest cell.  Returns content[core][slot] = (b, k0, klen)
    (b = -1 for empty cells) or None if infeasible."""
    cells = []
    for j, cap in enumerate(caps):
        for c in range(NCORES):
            cells.append((cap, c, j))
    avail = sorted(cells)
    content = [[(-1, 0, 0)] * len(caps) for _ in range(NCORES)]
    for b in np.argsort(-vl, kind="stable"):
        rem = int(vl[b])
        k0 = 0
        while rem > 0:
            if not avail:
                return None
            caps_list = [x[0] for x in avail]
            i = bisect.bisect_left(caps_list, rem)
            if i < len(avail):
                cap, c, j = avail.pop(i)
                take = rem
            else:
                cap, c, j = avail.pop()
                take = cap
            content[c][j] = (int(b), k0, take)
            k0 += take
            rem -= take
    return content


def _plan(valid_lens):
    """Search slot capacities minimizing padded work; returns
    (slots, content) with slots = tuple of V_j."""
    vl = np.asarray(valid_lens)
    cand = set()
    for v in vl:
        for k in (1, 2, 3, 4):
            cand.add(int(math.ceil(int(v) / k)))
    cand = sorted(x for x in cand if x >= 1)
    import itertools

    tot = int(vl.sum())
    best = None
    for S in (2, 3, 4):
        for caps in itertools.combinations_with_replacement(
            sorted(cand, reverse=True), S
        ):
            sv = sum(caps)
            if NCORES * sv < tot:
                continue
            if best is not None and Q * sv + S * 700.0 >= best[0]:
                continue
            content = _pack(vl, caps)
            if content is None:
                continue
            best = (Q * sv + S * 700.0, caps, content)
    caps, content = best[1], best[2]
    # Process the smallest slot first: its tiny first tanh fills the
    # ScalarE conveyor early while the big slots' inputs still stream in.
    order = sorted(range(len(caps)), key=lambda j: caps[j])
    order = [order[0]] + sorted(order[1:], key=lambda j: -caps[j])
    caps = tuple(caps[j] for j in order)
    content = [[row[j] for j in order] for row in content]
    return caps, content


def _slot_groups(s, V, dve_frac):
    """Partition a slot's V keys into (scalar_groups, dve_groups), each a
    list of (k0, Kg) in key order; scalar keys first."""
    if V <= 16:
        n_dve = 0
    else:
        n_dve = int(round(dve_frac * V / 8.0)) * 8
        n_dve = min(n_dve, V - 8)
    Vs = V - n_dve
    sg = []
    k0 = 0
    rem = Vs
    if s == 0 and Vs > 8:
        sg.append((0, 4))
        k0, rem = 4, Vs - 4
    while rem > 0:
        g = min(KT, rem)
        sg.append((k0, g))
        k0 += g
        rem -= g
    dg = []
    rem = n_dve
    while rem > 0:
        g = min(KT, rem)
        dg.append((k0, g))
        k0 += g
        rem -= g
    return sg, dg


def _build_nc(caps, dve_frac=DVE_FRAC, gp_groups=GP_GROUPS):
    """Build + finalize the single-core SPMD program for slot caps."""
    S = len(caps)
    nc = bacc.Bacc(None, target_bir_lowering=False, debug=False)

    qkT = nc.declare_dram_parameter("qkT", [S, 2, D, Q], BF16, isOutput=False)
    vals = nc.declare_dram_parameter("vals", [S, K, D], BF16, isOutput=False)
    wqk_d = nc.declare_dram_parameter("wqk", [2, D, H], BF16, isOutput=False)
    wv_d = nc.declare_dram_parameter("wv8", [128, 8], BF16, isOutput=False)
    mask_d = nc.declare_dram_parameter("mask", [S, 128, K], F32, isOutput=False)
    out_d = nc.declare_dram_parameter("out", [S, Q, D + 1], F32, isOutput=True)

    Tanh = mybir.ActivationFunctionType.Tanh
    Exp = mybir.ActivationFunctionType.Exp

    # the gp_groups biggest slots each get their LAST scalar add-group on
    # GpSimd (emitted early, right after that slot's projections)
    gp_slots = set(
        sorted(range(S), key=lambda j: -caps[j])[: max(0, gp_groups)]
    )

    with tile.TileContext(nc) as tc:
        with (
            tc.tile_pool(name="const", bufs=1) as constp,
            tc.tile_pool(name="io", bufs=1) as iop,
            tc.tile_pool(name="proj", bufs=1) as projp,
            tc.tile_pool(name="stage", bufs=3) as stagep,
            tc.tile_pool(name="sm", bufs=2) as smp,
            tc.tile_pool(name="ps_proj", bufs=3, space="PSUM") as ps_proj,
            tc.tile_pool(name="ps_sc", bufs=3, space="PSUM") as ps_sc,
            tc.tile_pool(name="ps_misc", bufs=1, space="PSUM") as ps_misc,
        ):
            # ---- constants & inputs (critical-path DMAs first) ----------
            wqk_sb = constp.tile([128, 2, 4, H], BF16, tag="wqk")
            wqk_r = wqk_d[:].rearrange("w (c p) h -> p w c h", p=128)
            qkt_sb = iop.tile([128, S, 2, 4, Q], BF16, tag="qkt")
            qkT_r = qkT[:].rearrange("s w (c p) x -> p s w c x", p=128)
            nc.sync.dma_start(wqk_sb[:], wqk_r[:])
            for s in range(S):
                nc.sync.dma_start(qkt_sb[:, s], qkT_r[:, s])
            wq_sb = wqk_sb[:, 0]
            wk_sb = wqk_sb[:, 1]
            qt_sb = qkt_sb[:, :, 0]
            kt_sb = qkt_sb[:, :, 1]
            wv_sb = constp.tile([128, 8], BF16, tag="wv")
            nc.sync.dma_start(wv_sb[:], wv_d[:])
            ident = constp.tile([128, 128], BF16, tag="ident")
            make_identity(nc, ident[:])
            vals_sb = iop.tile([128, S, D], BF16, tag="vals")
            nc.sync.dma_start(vals_sb[:], vals[:].rearrange("s k d -> k s d"))
            mask_sb = iop.tile([128, S, K], F32, tag="mask")
            nc.sync.dma_start(mask_sb[:], mask_d[:].rearrange("s p k -> p s k"))

            # ---- projections: projT[h,x] = sum_d W[d,h] * xT[d,x] -------
            # kproj2 holds each projected key DUPLICATED ([..., k, 2]) so
            # the scalar-path broadcast-add runs in DVE 2x_1P packed mode.
            qproj = projp.tile([128, S, 4, Q], BF16, tag="qproj")
            kproj2 = projp.tile([128, S, 4, K, 2], BF16, tag="kproj")

            # per-slot gp-add groups, filled by the slot loop; projections
            # emit the gp adds for slot s right after slot s's casts.
            slot_plan = {}
            for s in range(S):
                sg, dg = _slot_groups(s, caps[s], dve_frac)
                gp_idx = len(sg) - 1 if (s in gp_slots and len(sg) > 1) else -1
                slot_plan[s] = (sg, dg, gp_idx)

            def emit_add(s, k0, Kg, eng):
                pre = stagep.tile([128, 4, KT * Q], BF16, tag="pre")
                for hc in range(4):
                    in0 = (
                        kproj2[:, s, hc, k0 : k0 + Kg, :]
                        .unsqueeze(2)
                        .broadcast_to((128, Kg, Q // 2, 2))
                    )
                    in1 = (
                        qproj[:, s, hc, :]
                        .rearrange("p (qp j) -> p qp j", j=2)
                        .unsqueeze(1)
                        .broadcast_to((128, Kg, Q // 2, 2))
                    )
                    out = pre[:, hc, : Kg * Q].rearrange(
                        "p (kl qp j) -> p kl qp j", qp=Q // 2, j=2
                    )
                    eng.tensor_add(out, in0, in1)
                return pre

            def project(s):
                V = caps[s]
                for hc in range(4):
                    pq = ps_proj.tile([128, 128], F32, tag="pp", name=f"pq{s}_{hc}")
                    for dc in range(4):
                        nc.tensor.matmul(
                            pq[:],
                            wq_sb[:, dc, hc * 128 : (hc + 1) * 128],
                            qt_sb[:, s, dc, :],
                            start=(dc == 0),
                            stop=(dc == 3),
                        )
                    nc.vector.tensor_copy(qproj[:, s, hc, :], pq[:])
                    pk = ps_proj.tile([128, 128], F32, tag="pp", name=f"pk{s}_{hc}")
                    for dc in range(4):
                        nc.tensor.matmul(
                            pk[:, :V],
                            wk_sb[:, dc, hc * 128 : (hc + 1) * 128],
                            kt_sb[:, s, dc, :V],
                            start=(dc == 0),
                            stop=(dc == 3),
                        )
                    nc.vector.tensor_copy(
                        kproj2[:, s, hc, :V, :],
                        pk[:, :V].unsqueeze(2).broadcast_to((128, V, 2)),
                    )
                # early GpSimd add for this slot's designated group
                sg, dg, gp_idx = slot_plan[s]
                if gp_idx >= 0:
                    k0, Kg = sg[gp_idx]
                    slot_plan[s] = (sg, dg, gp_idx)
                    pre = emit_add(s, k0, Kg, nc.gpsimd)
                    _gp_pre[s] = pre

            _gp_pre = {}

            # persistent softmax state (cols >= V are never read into live
            # results: the output matmul contracts over eT[:V] only)
            e_sb = projp.tile([128, S, K], BF16, tag="e")

            # ---- epilogue (emitted one slot late, see baseline notes) ----
            def epilogue(s, psc):
                V = caps[s]
                msc = smp.tile([128, K], F32, tag="msc", name=f"msc{s}")
                nc.vector.tensor_add(msc[:, :V], psc[:, :V], mask_sb[:, s, :V])
                o_sb = smp.tile([128, D + 1], F32, tag="o", name=f"o{s}")
                nc.scalar.activation(e_sb[:, s, :V], msc[:, :V], Exp)
                nc.vector.tensor_reduce(
                    o_sb[:, D : D + 1],
                    e_sb[:, s, :V],
                    axis=mybir.AxisListType.X,
                    op=mybir.AluOpType.add,
                )
                pt = ps_misc.tile([128, 128], BF16, tag="pt", name=f"pt{s}")
                nc.tensor.transpose(pt[:], e_sb[:, s, :], ident[:])
                eT = smp.tile([128, 128], BF16, tag="eT", name=f"eT{s}")
                nc.vector.tensor_copy(eT[:], pt[:])
                po = ps_misc.tile([128, D], F32, tag="po", name=f"po{s}")
                nc.tensor.matmul(
                    po[:, :], eT[:V, :], vals_sb[:V, s, :], start=True, stop=True
                )
                nc.vector.tensor_copy(o_sb[:, :D], po[:])
                nc.sync.dma_start(out_d[s], o_sb[:])

            # ---- main loop ----------------------------------------------
            pending = None
            project(0)
            for s in range(S):
                V = caps[s]
                sg, dg, gp_idx = slot_plan[s]
                psc = ps_sc.tile([128, K], F32, tag="psc", name=f"psc{s}")
                prev_last = [None]

                def emit_scores(tnh3, k0, Kg, wbase):
                    for kl in range(Kg):
                        first = None
                        for hc in range(4):
                            bi = nc.tensor.matmul(
                                psc[:, k0 + kl : k0 + kl + 1],
                                tnh3[:, hc, kl, :],
                                wv_sb[:, wbase + hc : wbase + hc + 1],
                                start=(hc == 0),
                                stop=(hc == 3),
                            )
                            if hc == 0:
                                first = bi.ins
                            last = bi.ins
                        if prev_last[0] is not None:
                            tile.add_dep_helper(
                                first, prev_last[0], sync=False,
                                reason="psc accumulation-group order",
                            )
                        prev_last[0] = last

                # custom-DVE work queue for this slot: (tnh tile, hc, k0, Kg)
                dve_queue = []
                for k0, Kg in dg:
                    dtnh = stagep.tile([128, 4, KT * Q], BF16, tag="pre")
                    for hc in range(4):
                        dve_queue.append((dtnh, hc, k0, Kg))

                n_sg = max(1, len(sg))
                per = (len(dve_queue) + n_sg - 1) // n_sg

                def pop_dve(n):
                    for _ in range(n):
                        if not dve_queue:
                            return
                        dtnh, hc, k0, Kg = dve_queue.pop(0)
                        in0 = (
                            kproj2[:, s, hc, k0 : k0 + Kg, 0]
                            .unsqueeze(2)
                            .broadcast_to((128, Kg, Q))
                        )
                        in1 = (
                            qproj[:, s, hc, :]
                            .unsqueeze(1)
                            .broadcast_to((128, Kg, Q))
                        )
                        out = dtnh[:, hc, : Kg * Q].rearrange(
                            "p (kl q) -> p kl q", q=Q
                        )
                        nc.vector._custom_dve(
                            TANH_OP, out=out, in0=in0, in1=in1, s0=C0V, s1=C1V
                        )
                        if hc == 3:
                            tnh3 = dtnh[:, :, : Kg * Q].rearrange(
                                "p hc (kl q) -> p hc kl q", q=Q
                            )
                            emit_scores(tnh3, k0, Kg, 4)

                for g, (k0, Kg) in enumerate(sg):
                    nflat = Kg * Q
                    if g == gp_idx:
                        pre = _gp_pre[s]
                    else:
                        pre = emit_add(s, k0, Kg, nc.vector)
                    tnh = stagep.tile([128, 4, KT * Q], BF16, tag="tnh")
                    if s == 0 and g == 0:
                        # ramp: per-chunk tanh starts right after the first
                        # broadcast-add instead of after all four
                        for hc in range(4):
                            nc.scalar.activation(
                                tnh[:, hc, :nflat], pre[:, hc, :nflat], Tanh
                            )
                    else:
                        nc.scalar.activation(
                            tnh[:, :, :nflat], pre[:, :, :nflat], Tanh
                        )
                    pop_dve(per)
                    tnh3 = tnh[:, :, :nflat].rearrange(
                        "p hc (kl q) -> p hc kl q", q=Q
                    )
                    emit_scores(tnh3, k0, Kg, 0)
                    if g == 0 and s + 1 < S:
                        project(s + 1)
                    if g == min(1, len(sg) - 1) and pending is not None:
                        epilogue(*pending)
                        pending = None
                pop_dve(len(dve_queue))
                if pending is not None:
                    epilogue(*pending)
                pending = (s, psc)
            epilogue(*pending)

    nc.finalize()
    return nc


def kernel(queries, keys, values, valid_lens, Wq, Wk, wv):
    global LAST_RESULT
    queries = np.asarray(queries, dtype=np.float32)
    keys = np.asarray(keys, dtype=np.float32)
    values = np.asarray(values, dtype=np.float32)
    valid_lens = np.asarray(valid_lens, dtype=np.int32)
    Wq = np.asarray(Wq, dtype=np.float32)
    Wk = np.asarray(Wk, dtype=np.float32)
    wv = np.asarray(wv, dtype=np.float32)

    caps, content = _plan(valid_lens)
    S = len(caps)

    key = (caps, DVE_FRAC, GP_GROUPS)
    if key not in _NC_CACHE:
        _NC_CACHE[key] = _build_nc(caps, DVE_FRAC, GP_GROUPS)
    nc = _NC_CACHE[key]

    # ---- host-side shard prep -------------------------------------------
    wqk = np.stack([Wq, Wk]).astype(BF16_NP)
    wv4 = np.ascontiguousarray(wv.reshape(4, 128).T)  # [128,4] f32
    wv8 = np.concatenate([wv4, KV * wv4], axis=1).astype(BF16_NP)  # [128,8]
    qTt = {
        b: np.ascontiguousarray(queries[b].T).astype(BF16_NP) for b in range(B)
    }

    in_maps = []
    for c in range(NCORES):
        qkTm = np.zeros((S, 2, D, Q), dtype=BF16_NP)
        valsm = np.zeros((S, K, D), dtype=BF16_NP)
        maskm = np.zeros((S, 128, K), dtype=np.float32)
        for s, (b, k0, klen) in enumerate(content[c]):
            if b < 0:
                maskm[s, :, :] = NEG
                continue
            qkTm[s, 0] = qTt[b]
            qkTm[s, 1, :, :klen] = keys[b, k0 : k0 + klen].T.astype(BF16_NP)
            valsm[s, :klen] = values[b, k0 : k0 + klen].astype(BF16_NP)
            maskm[s, :, klen:] = NEG
        in_maps.append(
            {
                "qkT": qkTm,
                "vals": valsm,
                "wqk": wqk,
                "wv8": wv8,
                "mask": maskm,
            }
        )

    res = run_bass_kernel_spmd(nc, in_maps, list(range(NCORES)))
    LAST_RESULT = res

    O = np.zeros((B, Q, D), dtype=np.float64)
    Z = np.zeros((B, Q, 1), dtype=np.float64)
    for c in range(NCORES):
        oz = np.asarray(res.results[c]["out"], dtype=np.float64)
        for s, (b, k0, klen) in enumerate(content[c]):
            if b < 0:
                continue
            O[b] += oz[s, :, :D]
            Z[b] += oz[s, :, D:]
    return (O / Z).astype(np.float32)


# revision 9
# speedup vs baseline: 1.2287x; 1.2287x over previous
"""Trainium2 Bass kernel for nn_AdditiveAttention (Bahdanau attention).

Reference computation (B=16, Q=128, K=128, D=512, H=512):
    q = queries @ Wq                     [B,Q,H]
    k = keys @ Wk                        [B,K,H]
    scores[b,q,k] = sum_h wv[h] * tanh(q[b,q,h] + k[b,k,h])
    attn = softmax over valid keys (k < valid_lens[b])
    out = attn @ values                  [B,Q,D]

Strategy (8 NeuronCores, SPMD, key-split data parallelism):
  Each batch's valid key range is split into contiguous fragments packed
  into 8 cores x S uniform slots (see _plan/_pack).  A cell computes the
  UNNORMALIZED partial o = exp(scores) @ values and z = sum(exp(scores));
  the host combines out[b] = sum(o) / sum(z).

  The per-key work sum_h wv_h * tanh(qp_h + kp_h) is elementwise-bound
  (H*Q = 65536 elements per key).  To beat the single-engine limit the
  keys of each slot are SPLIT across two compute paths:
    * scalar path: broadcast-add on DVE (2x_1P packed bf16) or GpSimd,
      then exact tanh on ScalarE in big batched instructions.
    * dve path: ONE custom DVE instruction per (group, h-chunk) that
      fuses the broadcast add with a clamped-cubic tanh approximation
      f(u) = m*(1 + c1*m^2), m = clamp(u, +-c0); the global scale K of
      the fit K*f ~ tanh is folded into a second copy of wv.
  The wv reduction runs on TensorE (per key: 4 accumulated [128h x 128q]
  x [128h x 1] matmuls into one PSUM score column); masked exp on
  ScalarE; transpose + exp@values on TensorE.
"""

import os
import sys
import types
import math
import bisect
import numpy as np
import ml_dtypes

# ---------------------------------------------------------------------------
# axon NTFF profile hook (lets trace=True / BASS_TRACE=1 work in this image)
# ---------------------------------------------------------------------------
def _install_axon_hooks():
    if "antenv.axon_hooks" in sys.modules:
        return
    try:
        import trn_agent_boot.trn_boot as _tb

        _hooks = types.ModuleType("antenv.axon_hooks")
        _hook = _tb._ntff_profile_via_ctypes("/opt/axon/libaxon_pjrt.so")
        _hooks.get_axon_ntff_profile_hook = lambda: _hook
        _hooks.set_axon_ntff_profile_hook = lambda h: None
        sys.modules["antenv.axon_hooks"] = _hooks
    except Exception:
        pass


_install_axon_hooks()

import concourse.bass as bass
import concourse.bacc as bacc
import concourse.mybir as mybir
import concourse.tile as tile
import concourse.bass_utils as bass_utils
from concourse.bass_utils import run_bass_kernel_spmd
from concourse.masks import make_identity

# Avoid S3 artifact-upload attempts in the trace path.
bass_utils.upload_artifacts = lambda tmpdir: tmpdir

F32 = mybir.dt.float32
BF16 = mybir.dt.bfloat16
BF16_NP = ml_dtypes.bfloat16

B, Q, K, D, H = 16, 128, 128, 512, 512
NCORES = 8
KT = 16  # key-columns per tanh group
NEG = -1e9

# clamped-cubic tanh fit: tanh(u) ~ KV * m * (1 + C1V*m^2), m = clip(u, +-C0V)
# (density-weighted least squares against u ~ N(0, 1.42), the empirical
# distribution of q_h + k_h for this problem's scale)
C0V, C1V, KV = 1.59679101, -0.13073221, 0.89431216

# tunables
DVE_FRAC = 0.18   # fraction of each slot's keys on the custom-DVE tanh path
GP_GROUPS = 0     # scalar-path add-groups on GpSimd: DISABLED — concurrent
                  # GpSimd adds contend for the same SBUF tiles and slow the
                  # DVE adds 4x (measured 1135 -> 4545 ns per 2048-elem add)

_NC_CACHE: dict = {}
LAST_RESULT = None


# ---------------------------------------------------------------------------
# custom DVE op: fused broadcast-add + clamped-cubic tanh
# ---------------------------------------------------------------------------
def _register_tanh_op():
    import concourse.dve_ops as dve_ops
    from concourse.dve_ops import DveOp
    from concourse.dve_spec import Spec, Src0, Src1, C0, C1, Zero, One, maxx, minn
    from concourse.dve_spec import lower
    from concourse.dve_uop import DveOpSpec

    name = "TANH_BAHDANAU_ANT"
    if name in dve_ops._SUB_OPCODE_FOR_NAME:
        return next(op for op in dve_ops.OPS if op.name == name)
    u = Src0 + Src1
    m = maxx(minn(u, C0), Zero - C0)
    v = m * m
    body = m * ((v * C1) + One)

    def ref(in0, in1, s0, s1, imm2):
        mm = np.clip(in0.astype(np.float32) + in1, -s0, s0)
        return (mm * (1.0 + s1 * mm * mm)).astype(np.float32)

    spec = Spec(body=body, reference=ref)
    row = max(dve_ops._SUB_OPCODE_FOR_NAME.values()) + 1
    assert row < 0x20
    dve_ops._SUB_OPCODE_FOR_NAME[name] = row
    ver = "v3"
    tmp = DveOpSpec(name=name, opcode=row, uops=lower(spec, ver=ver), rd1_en=True)
    op = DveOp(name, spec, subdim=False, uops_sha={ver: tmp.sha(ver)})
    dve_ops.OPS.append(op)
    dve_ops.CUSTOM_DVE_SPECS[name] = spec
    return op


TANH_OP = _register_tanh_op()


def _pack(vl, caps):
    """Pack each batch's valid keys as contiguous ranges into cells (one
    range per cell).  Best-fit: smallest cell that fits the remainder,
    else the largest cell.  Returns content[core][slot] = (b, k0, klen)
    (b = -1 for empty cells) or None if infeasible."""
    cells = []
    for j, cap in enumerate(caps):
        for c in range(NCORES):
            cells.append((cap, c, j))
    avail = sorted(cells)
    content = [[(-1, 0, 0)] * len(caps) for _ in range(NCORES)]
    for b in np.argsort(-vl, kind="stable"):
        rem = int(vl[b])
        k0 = 0
        while rem > 0:
            if not avail:
                return None
            caps_list = [x[0] for x in avail]
            i = bisect.bisect_left(caps_list, rem)
            if i < len(avail):
                cap, c, j = avail.pop(i)
                take = rem
            else:
                cap, c, j = avail.pop()
                take = cap
            content[c][j] = (int(b), k0, take)
            k0 += take
            rem -= take
    return content


def _plan(valid_lens):
    """Search slot capacities minimizing padded work; returns
    (slots, content) with slots = tuple of V_j."""
    vl = np.asarray(valid_lens)
    cand = set()
    for v in vl:
        for k in (1, 2, 3, 4):
            cand.add(int(math.ceil(int(v) / k)))
    cand = sorted(x for x in cand if x >= 1)
    import itertools

    tot = int(vl.sum())
    best = None
    for S in (2, 3, 4):
        for caps in itertools.combinations_with_replacement(
            sorted(cand, reverse=True), S
        ):
            sv = sum(caps)
            if NCORES * sv < tot:
                continue
            if best is not None and Q * sv + S * 700.0 >= best[0]:
                continue
            content = _pack(vl, caps)
            if content is None:
                continue
            best = (Q * sv + S * 700.0, caps, content)
    caps, content = best[1], best[2]
    # Process the smallest slot first: its tiny first tanh fills the
    # ScalarE conveyor early while the big slots' inputs still stream in.
    order = sorted(range(len(caps)), key=lambda j: caps[j])
    order = [order[0]] + sorted(order[1:], key=lambda j: -caps[j])
    caps = tuple(caps[j] for j in order)
    content = [[row[j] for j in order] for row in content]
    return caps, content


def _slot_groups(s, V, dve_frac):
    """Partition a slot's V keys into (scalar_groups, dve_groups), each a
    list of (k0, Kg) in key order; scalar keys first."""
    if V <= 16:
        n_dve = 0
    else:
        n_dve = int(round(dve_frac * V / 8.0)) * 8
        n_dve = min(n_dve, V - 8)
    Vs = V - n_dve
    sg = []
    k0 = 0
    rem = Vs
    if s == 0 and Vs > 8:
        sg.append((0, 4))
        k0, rem = 4, Vs - 4
    while rem > 0:
        g = min(KT, rem)
        sg.append((k0, g))
        k0 += g
        rem -= g
    dg = []
    rem = n_dve
    while rem > 0:
        g = min(KT, rem)
        dg.append((k0, g))
        k0 += g
        rem -= g
    return sg, dg


def _build_nc(caps, dve_frac=DVE_FRAC, gp_groups=GP_GROUPS):
    """Build + finalize the single-core SPMD program for slot caps."""
    S = len(caps)
    nc = bacc.Bacc(None, target_bir_lowering=False, debug=False)

    qkT = nc.declare_dram_parameter("qkT", [S, 2, D, Q], BF16, isOutput=False)
    vals = nc.declare_dram_parameter("vals", [S, K, D], BF16, isOutput=False)
    wqk_d = nc.declare_dram_parameter("wqk", [2, D, H], BF16, isOutput=False)
    wv_d = nc.declare_dram_parameter("wv8", [128, 8], BF16, isOutput=False)
    mask_d = nc.declare_dram_parameter("mask", [S, 128, K], F32, isOutput=False)
    out_d = nc.declare_dram_parameter("out", [S, Q, D + 1], F32, isOutput=True)

    Tanh = mybir.ActivationFunctionType.Tanh
    Exp = mybir.ActivationFunctionType.Exp

    # the gp_groups biggest slots each get their LAST scalar add-group on
    # GpSimd (emitted early, right after that slot's projections)
    gp_slots = set(
        sorted(range(S), key=lambda j: -caps[j])[: max(0, gp_groups)]
    )

    with tile.TileContext(nc) as tc:
        with (
            tc.tile_pool(name="const", bufs=1) as constp,
            tc.tile_pool(name="io", bufs=1) as iop,
            tc.tile_pool(name="proj", bufs=1) as projp,
            tc.tile_pool(name="stage", bufs=3) as stagep,
            tc.tile_pool(name="sm", bufs=2) as smp,
            tc.tile_pool(name="ps_proj", bufs=3, space="PSUM") as ps_proj,
            tc.tile_pool(name="ps_sc", bufs=3, space="PSUM") as ps_sc,
            tc.tile_pool(name="ps_misc", bufs=1, space="PSUM") as ps_misc,
        ):
            # ---- constants & inputs (critical-path DMAs first) ----------
            wqk_sb = constp.tile([128, 2, 4, H], BF16, tag="wqk")
            wqk_r = wqk_d[:].rearrange("w (c p) h -> p w c h", p=128)
            qkt_sb = iop.tile([128, S, 2, 4, Q], BF16, tag="qkt")
            qkT_r = qkT[:].rearrange("s w (c p) x -> p s w c x", p=128)
            nc.sync.dma_start(wqk_sb[:], wqk_r[:])
            for s in range(S):
                nc.sync.dma_start(qkt_sb[:, s], qkT_r[:, s])
            wq_sb = wqk_sb[:, 0]
            wk_sb = wqk_sb[:, 1]
            qt_sb = qkt_sb[:, :, 0]
            kt_sb = qkt_sb[:, :, 1]
            wv_sb = constp.tile([128, 8], BF16, tag="wv")
            nc.sync.dma_start(wv_sb[:], wv_d[:])
            ident = constp.tile([128, 128], BF16, tag="ident")
            make_identity(nc, ident[:])
            vals_sb = iop.tile([128, S, D], BF16, tag="vals")
            nc.sync.dma_start(vals_sb[:], vals[:].rearrange("s k d -> k s d"))
            mask_sb = iop.tile([128, S, K], F32, tag="mask")
            nc.sync.dma_start(mask_sb[:], mask_d[:].rearrange("s p k -> p s k"))

            # ---- projections: projT[h,x] = sum_d W[d,h] * xT[d,x] -------
            # kproj2 holds each projected key DUPLICATED ([..., k, 2]) so
            # the scalar-path broadcast-add runs in DVE 2x_1P packed mode.
            qproj = projp.tile([128, S, 4, Q], BF16, tag="qproj")
            kproj2 = projp.tile([128, S, 4, K, 2], BF16, tag="kproj")

            # per-slot gp-add groups, filled by the slot loop; projections
            # emit the gp adds for slot s right after slot s's casts.
            slot_plan = {}
            for s in range(S):
                sg, dg = _slot_groups(s, caps[s], dve_frac)
                gp_idx = len(sg) - 1 if (s in gp_slots and len(sg) > 1) else -1
                slot_plan[s] = (sg, dg, gp_idx)

            def emit_add(s, k0, Kg, eng):
                pre = stagep.tile([128, 4, KT * Q], BF16, tag="pre")
                for hc in range(4):
                    in0 = (
                        kproj2[:, s, hc, k0 : k0 + Kg, :]
                        .unsqueeze(2)
                        .broadcast_to((128, Kg, Q // 2, 2))
                    )
                    in1 = (
                        qproj[:, s, hc, :]
                        .rearrange("p (qp j) -> p qp j", j=2)
                        .unsqueeze(1)
                        .broadcast_to((128, Kg, Q // 2, 2))
                    )
                    out = pre[:, hc, : Kg * Q].rearrange(
                        "p (kl qp j) -> p kl qp j", qp=Q // 2, j=2
                    )
                    eng.tensor_add(out, in0, in1)
                return pre

            def project(s):
                V = caps[s]
                for hc in range(4):
                    pq = ps_proj.tile([128, 128], F32, tag="pp", name=f"pq{s}_{hc}")
                    for dc in range(4):
                        nc.tensor.matmul(
                            pq[:],
                            wq_sb[:, dc, hc * 128 : (hc + 1) * 128],
                            qt_sb[:, s, dc, :],
                            start=(dc == 0),
                            stop=(dc == 3),
                        )
                    nc.vector.tensor_copy(qproj[:, s, hc, :], pq[:])
                    pk = ps_proj.tile([128, 128], F32, tag="pp", name=f"pk{s}_{hc}")
                    for dc in range(4):
                        nc.tensor.matmul(
                            pk[:, :V],
                            wk_sb[:, dc, hc * 128 : (hc + 1) * 128],
                            kt_sb[:, s, dc, :V],
                            start=(dc == 0),
                            stop=(dc == 3),
                        )
                    nc.vector.tensor_copy(
                        kproj2[:, s, hc, :V, :],
                        pk[:, :V].unsqueeze(2).broadcast_to((128, V, 2)),
                    )
                # early GpSimd add for this slot's designated group
                sg, dg, gp_idx = slot_plan[s]
                if gp_idx >= 0:
                    k0, Kg = sg[gp_idx]
                    slot_plan[s] = (sg, dg, gp_idx)
                    pre = emit_add(s, k0, Kg, nc.gpsimd)
                    _gp_pre[s] = pre

            _gp_pre = {}

            # persistent softmax state (cols >= V are never read into live
            # results: the output matmul contracts over eT[:V] only)
            e_sb = projp.tile([128, S, K], BF16, tag="e")

            # ---- epilogue (emitted one slot late, see baseline notes) ----
            def epilogue(s, psc):
                V = caps[s]
                msc = smp.tile([128, K], F32, tag="msc", name=f"msc{s}")
                nc.vector.tensor_add(msc[:, :V], psc[:, :V], mask_sb[:, s, :V])
                o_sb = smp.tile([128, D + 1], F32, tag="o", name=f"o{s}")
                nc.scalar.activation(e_sb[:, s, :V], msc[:, :V], Exp)
                nc.vector.tensor_reduce(
                    o_sb[:, D : D + 1],
                    e_sb[:, s, :V],
                    axis=mybir.AxisListType.X,
                    op=mybir.AluOpType.add,
                )
                pt = ps_misc.tile([128, 128], BF16, tag="pt", name=f"pt{s}")
                nc.tensor.transpose(pt[:], e_sb[:, s, :], ident[:])
                eT = smp.tile([128, 128], BF16, tag="eT", name=f"eT{s}")
                nc.vector.tensor_copy(eT[:], pt[:])
                po = ps_misc.tile([128, D], F32, tag="po", name=f"po{s}")
                nc.tensor.matmul(
                    po[:, :], eT[:V, :], vals_sb[:V, s, :], start=True, stop=True
                )
                nc.vector.tensor_copy(o_sb[:, :D], po[:])
                nc.sync.dma_start(out_d[s], o_sb[:])

            # ---- main loop ----------------------------------------------
            pending = None
            project(0)
            for s in range(S):
                V = caps[s]
                sg, dg, gp_idx = slot_plan[s]
                psc = ps_sc.tile([128, K], F32, tag="psc", name=f"psc{s}")
                prev_last = [None]

                def emit_scores(tnh3, k0, Kg, wbase):
                    for kl in range(Kg):
                        first = None
                        for hc in range(4):
                            bi = nc.tensor.matmul(
                                psc[:, k0 + kl : k0 + kl + 1],
                                tnh3[:, hc, kl, :],
                                wv_sb[:, wbase + hc : wbase + hc + 1],
                                start=(hc == 0),
                                stop=(hc == 3),
                            )
                            if hc == 0:
                                first = bi.ins
                            last = bi.ins
                        if prev_last[0] is not None:
                            tile.add_dep_helper(
                                first, prev_last[0], sync=False,
                                reason="psc accumulation-group order",
                            )
                        prev_last[0] = last

                # custom-DVE work queue for this slot: (tnh tile, hc, k0, Kg)
                dve_queue = []
                for k0, Kg in dg:
                    dtnh = stagep.tile([128, 4, KT * Q], BF16, tag="pre")
                    for hc in range(4):
                        dve_queue.append((dtnh, hc, k0, Kg))

                n_sg = max(1, len(sg))
                per = (len(dve_queue) + n_sg - 1) // n_sg

                def pop_dve(n):
                    for _ in range(n):
                        if not dve_queue:
                            return
                        dtnh, hc, k0, Kg = dve_queue.pop(0)
                        in0 = (
                            kproj2[:, s, hc, k0 : k0 + Kg, 0]
                            .unsqueeze(2)
                            .broadcast_to((128, Kg, Q))
                        )
                        in1 = (
                            qproj[:, s, hc, :]
                            .unsqueeze(1)
                            .broadcast_to((128, Kg, Q))
                        )
                        out = dtnh[:, hc, : Kg * Q].rearrange(
                            "p (kl q) -> p kl q", q=Q
                        )
                        nc.vector._custom_dve(
                            TANH_OP, out=out, in0=in0, in1=in1, s0=C0V, s1=C1V
                        )
                        if hc == 3:
                            tnh3 = dtnh[:, :, : Kg * Q].rearrange(
                                "p hc (kl q) -> p hc kl q", q=Q
                            )
                            emit_scores(tnh3, k0, Kg, 4)

                for g, (k0, Kg) in enumerate(sg):
                    nflat = Kg * Q
                    if g == gp_idx:
                        pre = _gp_pre[s]
                    else:
                        pre = emit_add(s, k0, Kg, nc.vector)
                    tnh = stagep.tile([128, 4, KT * Q], BF16, tag="tnh")
                    if s == 0 and g == 0:
                        # ramp: per-chunk tanh starts right after the first
                        # broadcast-add instead of after all four
                        for hc in range(4):
                            nc.scalar.activation(
                                tnh[:, hc, :nflat], pre[:, hc, :nflat], Tanh
                            )
                    else:
                        nc.scalar.activation(
                            tnh[:, :, :nflat], pre[:, :, :nflat], Tanh
                        )
                    pop_dve(per)
                    tnh3 = tnh[:, :, :nflat].rearrange(
                        "p hc (kl q) -> p hc kl q", q=Q
                    )
                    emit_scores(tnh3, k0, Kg, 0)
                    if g == 0 and s + 1 < S:
                        project(s + 1)
                    if g == min(1, len(sg) - 1) and pending is not None:
                        epilogue(*pending)
                        pending = None
                pop_dve(len(dve_queue))
                if pending is not None:
                    epilogue(*pending)
                pending = (s, psc)
            epilogue(*pending)

    nc.finalize()
    return nc


def kernel(queries, keys, values, valid_lens, Wq, Wk, wv):
    global LAST_RESULT
    queries = np.asarray(queries, dtype=np.float32)
    keys = np.asarray(keys, dtype=np.float32)
    values = np.asarray(values, dtype=np.float32)
    valid_lens = np.asarray(valid_lens, dtype=np.int32)
    Wq = np.asarray(Wq, dtype=np.float32)
    Wk = np.asarray(Wk, dtype=np.float32)
    wv = np.asarray(wv, dtype=np.float32)

    caps, content = _plan(valid_lens)
    S = len(caps)

    key = (caps, DVE_FRAC, GP_GROUPS)
    if key not in _NC_CACHE:
        _NC_CACHE[key] = _build_nc(caps, DVE_FRAC, GP_GROUPS)
    nc = _NC_CACHE[key]

    # ---- host-side shard prep -------------------------------------------
    wqk = np.stack([Wq, Wk]).astype(BF16_NP)
    wv4 = np.ascontiguousarray(wv.reshape(4, 128).T)  # [128,4] f32
    wv8 = np.concatenate([wv4, KV * wv4], axis=1).astype(BF16_NP)  # [128,8]
    qTt = {
        b: np.ascontiguousarray(queries[b].T).astype(BF16_NP) for b in range(B)
    }

    in_maps = []
    for c in range(NCORES):
        qkTm = np.zeros((S, 2, D, Q), dtype=BF16_NP)
        valsm = np.zeros((S, K, D), dtype=BF16_NP)
        maskm = np.zeros((S, 128, K), dtype=np.float32)
        for s, (b, k0, klen) in enumerate(content[c]):
            if b < 0:
                maskm[s, :, :] = NEG
                continue
            qkTm[s, 0] = qTt[b]
            qkTm[s, 1, :, :klen] = keys[b, k0 : k0 + klen].T.astype(BF16_NP)
            valsm[s, :klen] = values[b, k0 : k0 + klen].astype(BF16_NP)
            maskm[s, :, klen:] = NEG
        in_maps.append(
            {
                "qkT": qkTm,
                "vals": valsm,
                "wqk": wqk,
                "wv8": wv8,
                "mask": maskm,
            }
        )

    res = run_bass_kernel_spmd(nc, in_maps, list(range(NCORES)))
    LAST_RESULT = res

    O = np.zeros((B, Q, D), dtype=np.float64)
    Z = np.zeros((B, Q, 1), dtype=np.float64)
    for c in range(NCORES):
        oz = np.asarray(res.results[c]["out"], dtype=np.float64)
        for s, (b, k0, klen) in enumerate(content[c]):
            if b < 0:
                continue
            O[b] += oz[s, :, :D]
            Z[b] += oz[s, :, D:]
    return (O / Z).astype(np.float32)
